# revision 54
# baseline (speedup 1.0000x reference)
"""Conv2d(128->256, 3x3, pad=1) + sync-BatchNorm(train) + ReLU on 8 TRN2 cores.

Strategy (data-parallel, hardcoded for x:[32,128,56,56] w:[256,128,3,3]):
  - Shard batch 32 -> 4 images/core across 8 cores.
  - Host pre-pads x to 58x58 (fp16) and pre-transposes weights to
    [Cin, o_tile, tap, o] (fp16, contiguous 128-col weight slices -> FWL).
  - Conv = implicit GEMM: Cin=128 is the partition/contraction dim; each 3x3 tap
    is one fp16 matmul ([128,128] weights x [128,448] shifted-image view)
    accumulated in fp32 PSUM. Output rows in 7 groups of 8 rows (8*56=448).
  - BN train-mode: conv bias cancels exactly under BN; per-channel stats via
    DVE bn_stats on PSUM (count/mean/M2 in one pass) + bn_aggr.
  - Two-stage pipeline over the Cout halves: o=0's stats AllReduce, normalize
    and store overlap with o=1's matmuls; only o=1's tail is exposed.
  - Final: out = Relu(y*scale + shift) on ACT into an fp32 staging tile,
    DMA'd to DRAM.
"""

import numpy as np

import concourse.bass as bass
import concourse.mybir as mybir
import concourse.tile as tile
from concourse import bacc

F32 = mybir.dt.float32
F16 = mybir.dt.float16

N_CORES = 8
IMGS = 4            # images per core
CIN = 128
COUT = 256
H = W = 56
HP = WP = 58        # padded
NG = 7              # row-groups per image (8 rows each)
RG = 8              # rows per group
GROUP = RG * W      # 448
BANK = 512          # fp32 elems per PSUM bank
EPS = 1e-5

AF = mybir.ActivationFunctionType
ALU = mybir.AluOpType


def build_nc() -> bass.Bass:
    # Bacc (not raw Bass): its compile pipeline legalizes semaphore waits
    # (TRN2 allows at most one wait per instruction; matmul waits move to
    # ldweights / event-semaphore instructions).
    nc = bacc.Bacc()
    xp_d = nc.declare_dram_parameter("xp", [IMGS, CIN, HP, WP], F16, isOutput=False)
    wt_d = nc.declare_dram_parameter("wt", [CIN, 2, 9, 128], F16, isOutput=False)
    gb_d = nc.declare_dram_parameter("gb", [128, 4], F32, isOutput=False)
    out_d = nc.declare_dram_parameter("out", [IMGS, COUT, H, W], F32, isOutput=True)

    import os as _os
    no_ar = bool(_os.environ.get("CONVACT_NO_AR"))
    # default: CC collective (reliable).  CONVACT_RDMA_AR=1 opts into the
    # hand-rolled remote-DMA all-gather (faster tail when it works).
    cc_ar = not (no_ar or _os.environ.get("CONVACT_RDMA_AR"))
    fake_incs: list = []

    with tile.TileContext(nc) as tc:
        with (
            tc.tile_pool(name="const", bufs=1) as cpool,
            tc.tile_pool(name="psum", bufs=2, space="PSUM") as ppool,
            tc.tile_pool(name="ostg", bufs=2) as opool,
            tc.tile_pool(name="dram", bufs=1, space="DRAM") as dpool,
        ):
            Wt0 = cpool.tile([128, 9, 128], F16)
            Wt1 = cpool.tile([128, 9, 128], F16)
            Wts = [Wt0, Wt1]
            GB = cpool.tile([128, 4], F32)
            X = cpool.tile([128, IMGS, HP, WP], F16)
            Y = cpool.tile([128, 2, IMGS, NG, GROUP], F16)
            S6 = cpool.tile([128, 2, IMGS * NG, 6], F32)
            MV = cpool.tile([128, 2, 2], F32)
            P = cpool.tile([128, 2, 2], F32)
            G = cpool.tile([128, 2, 2], F32)
            sqm = cpool.tile([128, 2], F32)
            e8 = cpool.tile([128, 2], F32)
            v64 = cpool.tile([128, 2], F32)
            std8 = cpool.tile([128, 2], F32)
            inv = cpool.tile([128, 2], F32)
            sc = cpool.tile([128, 2], F32)
            sh = cpool.tile([128, 2], F32)
            t2 = cpool.tile([128, 2], F32)
            eps64T = cpool.tile([128, 1], F32)
            dummy = cpool.tile([128, 1], F32)
            arin0 = dpool.tile([128, 2], F32)
            arin1 = dpool.tile([128, 2], F32)
            arout0 = dpool.tile([128, 2], F32)
            arout1 = dpool.tile([128, 2], F32)
            arin = [arin0, arin1]
            arout = [arout0, arout1]
            # hand-rolled all-gather landing buffers + semaphores
            AG0 = cpool.tile([128, 8, 2], F32)
            AG1 = cpool.tile([128, 8, 2], F32)
            AGs = [AG0, AG1]
            R40 = cpool.tile([128, 4, 2], F32)
            R41 = cpool.tile([128, 4, 2], F32)
            R20 = cpool.tile([128, 2, 2], F32)
            R21 = cpool.tile([128, 2, 2], F32)
            R4s, R2s = [R40, R41], [R20, R21]
            ag_rsem = [nc.alloc_semaphore(name="ag_rs0"), nc.alloc_semaphore(name="ag_rs1")]
            ag_lsem = nc.alloc_semaphore(name="ag_ls")
            ag_psem = nc.alloc_semaphore(name="ag_ps")

            # ---- loads: Wt(o=0) and the two X0 halves race on the three
            # trigger queues; everything else streams behind on sync ----
            nc.sync.dma_start(Wt0[:, 0:3, :], wt_d[:, 0, 0:3, :])
            # first tap-group of chunk A reads image-0 rows 0..33: race the
            # two halves on the scalar/gpsimd trigger queues, defer the rest
            nc.scalar.dma_start(X[:, 0, 0:17, :], xp_d[0, :, 0:17, :])
            nc.gpsimd.dma_start(X[:, 0, 17:34, :], xp_d[0, :, 17:34, :])
            nc.sync.dma_start(Wt0[:, 3:9, :], wt_d[:, 0, 3:9, :])
            nc.scalar.dma_start(X[:, 0, 34:HP, :], xp_d[0, :, 34:HP, :])
            nc.sync.dma_start(Wt1[:, :, :], wt_d[:, 1, :, :])
            nc.sync.dma_start(GB[:, :], gb_d[:, :])
            for n in range(1, IMGS):
                nc.sync.dma_start(X[:, n, :, :], xp_d[n, :, :, :])

            # warm the ACT table set that holds Rsqrt (Copy/Relu are fillers
            # in every set) so no table load lands mid-kernel
            nc.vector.memset(eps64T[:, :], 64.0 * EPS)
            nc.scalar.activation(dummy[:, :], eps64T[:, :], AF.Sqrt)

            # 4+2+1 PSUM-bank chunks: the last chunk is a single group so
            # only ~2.1us of stats-close separates the final matmul from the
            # tail AllReduce launch (was 3.4us with a 3-group last chunk)
            chunks = [(0, 4), (4, 2), (6, 1)]

            def conv_chunk(o, n, g0, ngr):
                ps = ppool.tile([128, 4, BANK], F32, tag="ps")
                # taps outer: consecutive matmuls share the stationary weights
                for t in range(9):
                    kh, kw = divmod(t, 3)
                    for gg in range(ngr):
                        g = g0 + gg
                        rhs = X[:, n, g * RG + kh : g * RG + kh + RG, kw : kw + W]
                        nc.tensor.matmul(
                            ps[:, gg, 0:GROUP],
                            Wts[o][:, t, :],
                            rhs,
                            start=(t == 0),
                            stop=(t == 8),
                        )
                # stats on DVE (they gate the AllReduces; DVE does nothing
                # else so it tracks the MM stream closely), evac on ACT
                for gg in range(ngr):
                    nc.vector.bn_stats(
                        S6[:, o, n * NG + g0 + gg, :], ps[:, gg, 0:GROUP]
                    )
                nc.scalar.activation(
                    Y[:, o, n, g0 : g0 + ngr, :], ps[:, 0:ngr, 0:GROUP], AF.Copy
                )

            def launch_ar(o, mid_hook=None):
                # per-core (mean, E[y^2]) packed into P[:,o]
                nc.vector.bn_aggr(MV[:, o, :], S6[:, o, :, :])
                nc.vector.tensor_mul(t2[:, o : o + 1], MV[:, o, 0:1], MV[:, o, 0:1])
                nc.vector.tensor_add(P[:, o, 1:2], MV[:, o, 1:2], t2[:, o : o + 1])
                nc.vector.tensor_copy(P[:, o, 0:1], MV[:, o, 0:1])
                if no_ar:
                    nc.vector.tensor_scalar_mul(P[:, o, :], P[:, o, :], float(N_CORES))
                    nc.gpsimd.dma_start(arin[o][:, :], P[:, o, :])
                    nc.gpsimd.dma_start(arout[o][:, :], arin[o][:, :])
                    nc.gpsimd.dma_start(G[:, o, :], arout[o][:, :])
                elif cc_ar:
                    nc.gpsimd.dma_start(arin[o][:, :], P[:, o, :])
                    nc.gpsimd.collective_compute(
                        "AllReduce",
                        ALU.add,
                        replica_groups=[list(range(N_CORES))],
                        ins=[arin[o].opt()],
                        outs=[arout[o].opt()],
                    )
                    # bulk stores queued here serialize BEHIND the tiny
                    # collective input on the gpsimd DMA queue, then stream
                    # during the collective's peer wait
                    if mid_hook is not None:
                        mid_hook()
                    nc.gpsimd.dma_start(G[:, o, :], arout[o][:, :])
                else:
                    # XOR all-gather: core c sends its payload to peer c^d
                    # into slot d (d=1..7); slot 0 is written locally.  Every
                    # receiver thus collects all 8 payloads (in XOR order --
                    # irrelevant, we only sum them); each remote arrival bumps
                    # remote_sem by 2 -> wait for 14.  The local slot-0 write
                    # also anchors the reduce in the Tile schedule (the
                    # arrival wait itself is invisible to the scheduler).
                    nc.vector.tensor_copy(AGs[o][:, 0, :], P[:, o, :])
                    # two batches (4+3): a single batch of 7 preps made the
                    # scheduler slip the 7th prep past the trigger (observed),
                    # while per-prep triggers hung NRT; 6-prep batches were
                    # observed to stay ordered
                    for batch in ((1, 2, 3, 4), (5, 6, 7)):
                        for d in batch:
                            rd: list = [None] * 8
                            rd[d] = (0, d)
                            nc.gpsimd.remote_dma_broadcast(
                                AGs[o][:, d, :],
                                P[:, o, :],
                                remote_sem=ag_rsem[o],
                                local_sem=ag_lsem,
                                rdests=rd,
                            )
                        nc.gpsimd.trigger_dma(count=None)
                    # Tile's single-core scheduling sim cannot see the remote
                    # increments that satisfy the arrival wait; give it a fake
                    # local inc that is stripped from the instruction stream
                    # after scheduling (build_nc removes it before finalize).
                    fake_incs.append(nc.gpsimd.sem_inc(ag_rsem[o], 14))

            def reduce_ag(o, eng):
                # sum the 8 gathered payloads -> G[:,o] = (sum mean, sum E2)
                AG, R4, R2 = AGs[o], R4s[o], R2s[o]
                first = eng.tensor_add(R4[:, :, :], AG[:, 0:4, :], AG[:, 4:8, :])
                first.wait_op(ag_rsem[o], 14, "sem-ge")
                eng.tensor_add(R2[:, :, :], R4[:, 0:2, :], R4[:, 2:4, :])
                eng.tensor_add(G[:, o, :], R2[:, 0, :], R2[:, 1, :])

            def finalize_a(o, eng=None):
                # G[:,o] = (sum_c mean_c, sum_c E2_c);  var*64 = 8*sumE2 - summean^2
                # o=0 runs on GpSimd: it is already serialized behind the AR
                # readback, so the AR-wait never blocks the busy DVE/ACT queues.
                # o=1 runs on the (by then idle) faster DVE.
                eng = eng or nc.gpsimd
                if not (no_ar or cc_ar):
                    reduce_ag(o, eng)
                eng.tensor_mul(sqm[:, o : o + 1], G[:, o, 0:1], G[:, o, 0:1])
                eng.tensor_scalar_mul(e8[:, o : o + 1], G[:, o, 1:2], 8.0)
                eng.tensor_sub(v64[:, o : o + 1], e8[:, o : o + 1], sqm[:, o : o + 1])
                # sqrt(64*var + 64*eps) = 8 * sqrt(var+eps)
                nc.scalar.activation(
                    std8[:, o : o + 1], v64[:, o : o + 1], AF.Sqrt, bias=eps64T[:, 0:1]
                )

            def finalize_b(o):
                nc.vector.reciprocal(inv[:, o : o + 1], std8[:, o : o + 1])
                # GB holds 8*gamma -> sc = (8*gamma)/(8*std) = gamma*rsqrt(var+eps)
                nc.vector.tensor_mul(
                    sc[:, o : o + 1], GB[:, o : o + 1], inv[:, o : o + 1]
                )
                # sh = beta - mean*sc;  mean = summean/8
                nc.vector.tensor_mul(t2[:, o : o + 1], G[:, o, 0:1], sc[:, o : o + 1])
                nc.vector.tensor_scalar_mul(t2[:, o : o + 1], t2[:, o : o + 1], -0.125)
                nc.vector.tensor_add(sh[:, o : o + 1], GB[:, 2 + o : 3 + o], t2[:, o : o + 1])

            def relu_part(o, n, halves=False):
                # halves=True: two [128, 1568] tiles so the first output DMA
                # fires ~1.4us after the scale/shift land (tail only)
                dst = out_d[n, o * 128 : (o + 1) * 128, :, :].rearrange(
                    "p h w -> p (h w)"
                )
                src = Y[:, o, n, :, :].rearrange("p a b -> p (a b)")
                # tail: quarter tile first (output DMA fires sooner), then
                # quarter + half
                bounds = [0, 784, 1568, H * W] if halves else [0, H * W]
                out = []
                for k in range(len(bounds) - 1):
                    lo, hi = bounds[k], bounds[k + 1]
                    ob = opool.tile([128, H * W], F32, tag="ob")
                    nc.scalar.activation(
                        ob[:, 0 : hi - lo],
                        src[:, lo:hi],
                        AF.Relu,
                        bias=sh[:, o : o + 1],
                        scale=sc[:, o : o + 1],
                    )
                    out.append((dst[:, lo:hi], ob[:, 0 : hi - lo]))
                return out

            def relu_store(o, n, halves=False, eng=None):
                eng = eng or nc.sync
                for dst, ob in relu_part(o, n, halves):
                    eng.dma_start(dst, ob)

            # ---- o=0 conv, then launch its AllReduce ----
            for n in range(IMGS):
                for g0, ngr in chunks:
                    conv_chunk(0, n, g0, ngr)
            launch_ar(0)

            # ---- o=1 conv (pure: no AR-dependent op sits in front of the
            # evac/stats work in any busy engine queue) ----
            for n in range(IMGS):
                for g0, ngr in chunks:
                    conv_chunk(1, n, g0, ngr)

            # o=0 finalize + normalize + store: overlaps the last part of the
            # o=1 matmul phase and hides AR#2's peer-skew wait
            finalize_a(0)
            finalize_b(0)
            o0_stores = []
            for n in range(IMGS):
                o0_stores.extend(relu_part(0, n))

            def _store_o0():
                for dst, ob in o0_stores:
                    nc.gpsimd.dma_start(dst, ob)

            launch_ar(1, mid_hook=_store_o0 if cc_ar else None)
            if not cc_ar:
                _store_o0()

            # ---- o=1 finalize + normalize + store (exposed tail) ----
            finalize_a(1, eng=nc.vector)
            finalize_b(1)
            for n in range(IMGS):
                relu_store(1, n, halves=True)

    # Dedup redundant LDWEIGHTS: taps-outer ordering makes up to 4
    # consecutive matmuls share the same stationary weights, but Tile emits an
    # Ldweights per matmul.  Removing the repeats (identical weight AP, no
    # attached wait, no different load in between) keeps the PE truly
    # back-to-back, avoiding the HAM micro-idle re-throttle between matmuls.
    # Matmuls never clobber the weight registers, so the previous load stays
    # valid.  (Only removals survive into the NEFF; reorders do not.)
    n_ldw_removed = 0
    for fn in nc.m.functions:
        for bb in fn.blocks:
            last_key = None
            to_remove = []
            for inst in bb.instructions:
                tn = type(inst).__name__
                if tn == "InstLdweights":
                    ap = inst.ins[0]
                    key = (ap.memref, ap.offset, str(ap.ap))
                    if key == last_key and not inst.has_wait():
                        to_remove.append(inst)
                    else:
                        last_key = key
            for inst in to_remove:
                bb.instructions.remove(inst)
                n_ldw_removed += 1
    assert n_ldw_removed == 0 or n_ldw_removed > 200, n_ldw_removed

    # strip the scheduler-only fake sem_incs: on hardware the arrival waits
    # must be satisfied by the 8 peers' remote DMA increments alone
    if fake_incs:
        names = set()
        for fi in fake_incs:
            inst = getattr(fi, "ins", fi)
            names.add(inst.name)
        removed = 0
        for fn in nc.m.functions:
            for bb in fn.blocks:
                for inst in list(bb.instructions):
                    if inst.name in names:
                        bb.instructions.remove(inst)
                        removed += 1
        assert removed == len(names), f"stripped {removed} != {len(names)}"
    return nc


_CACHE: dict = {}


def _get_nc() -> bass.Bass:
    if "nc" not in _CACHE:
        nc = build_nc()
        # Bacc.finalize runs the compile pipeline (wait legalization, register
        # allocation, nop fusion) - required before handing BIR to walrus.
        nc.finalize()
        _CACHE["nc"] = nc
    return _CACHE["nc"]


def _prep_inputs(x, weight, gamma, beta):
    x = np.asarray(x, dtype=np.float32)
    w = np.asarray(weight, dtype=np.float32)
    gamma = np.asarray(gamma, dtype=np.float32)
    beta = np.asarray(beta, dtype=np.float32)

    B = x.shape[0]
    per = B // N_CORES
    xp = np.zeros((B, CIN, HP, WP), np.float16)
    xp[:, :, 1 : 1 + H, 1 : 1 + W] = x
    # [Cout,Cin,3,3] -> [Cin, tap, Cout] -> [Cin, tap, o, 128] -> [Cin, o, tap, 128]
    wt = np.ascontiguousarray(
        w.transpose(1, 2, 3, 0).reshape(CIN, 9, 2, 128).transpose(0, 2, 1, 3),
        dtype=np.float16,
    )
    gb = np.ascontiguousarray(
        np.stack(
            [8.0 * gamma[:128], 8.0 * gamma[128:], beta[:128], beta[128:]], axis=1
        ),
        dtype=np.float32,
    )
    return [
        {"xp": xp[c * per : (c + 1) * per], "wt": wt, "gb": gb}
        for c in range(N_CORES)
    ]


def run(x, weight, bias=None, gamma=None, beta=None, trace=False, **kw):
    """Full-input entry; returns (out, BassKernelResults)."""
    from concourse.bass_utils import run_bass_kernel_spmd

    in_maps = _prep_inputs(x, weight, gamma, beta)
    res = run_bass_kernel_spmd(
        _get_nc(), in_maps, list(range(N_CORES)), trace=trace, **kw
    )
    out = np.concatenate([res.results[c]["out"] for c in range(N_CORES)], axis=0)
    return out, res


def kernel(x, weight, bias=None, gamma=None, beta=None):
    out, _ = run(x, weight, bias=bias, gamma=gamma, beta=beta, trace=False)
    return out
